# revision 1
# baseline (speedup 1.0000x reference)
"""Bass/Trainium2 kernel for elementwise Bessel J2 (nn_BesselFunction).

Input:  x float32 [64, 1048576], values in [0, 30)
Output: J2(x) float32 [64, 1048576]  (matches the NR-rational reference to
        ~2e-6 absolute, the reference's own fp32 noise floor)

Sharding: trivially data-parallel; row-block shard across 8 NeuronCores.
Each core sees a [128, 65536] view of its 8-row slice.

Math (per element, all fp32, ALL on the vector engine via fused custom ops —
the ACT table-based scalar engine and GPSIMD measured 7-15x slower than
modeled on this runtime, so everything lives on DVE):
  small (x < 8):  J2 ~ P9(t), t = x^2/32 - 1  (minimax fit, err 7e-8)
  big  (x >= 8):  J2 = rsqrt(x) * K(u) * sin(theta)
                  theta = x - 3pi/4 + r*g(u) - 2pi*round(...)  (Cody-Waite)
                  r = 1/x (1-instr approx), u = r^2
                  rsqrt via quadratic seed + 2 Newton steps
                  sin via odd deg-15 minimax poly on [-pi, pi]
  Select via uint8 mask + copy_predicated. Dead lanes (x<8 in the big path)
  may hold Inf/NaN; they are never read.
"""

import os
import sys

import numpy as np

for _p in ("/opt/trn_rl_repo", os.path.expanduser("~/.axon_site/_ro/trn_rl_repo")):
    if os.path.isdir(_p) and _p not in sys.path:
        sys.path.insert(0, _p)

# ---------------------------------------------------------------- constants
# Small branch: chain constants for deg-9 minimax poly in t = y/32-1
# (POLY3T top + 3x HORNER2), fitted to the reference's small-x rational.
SM_A0, SM_A1, SM_A2 = 0.00040356042792637144, -0.003646620070718724, 0.02440496663887849
SM_CH = (
    (-0.12099835224029, 0.39798150845136904, 0.0),
    (-0.7212394685922691, 0.2888979641713562, 0.0),
    (0.9515060387870121, -0.7681838588920249, -0.16211739076964357),
)
# Big branch: phase g(u) = g0 + g1*u + g2*u^2  (theta += r*g), amplitude
# K(u) = k0 + k1*u + k2*u^2 + k3*u^3
G0, G1, G2 = 1.8750017212985988, -0.35273547793587845, -1.269081537467201
K0, K1, K2, K3 = 0.7978845445579398, 0.7480321615210888, 0.627107329862385, -2.0328642098636296
BIAS_3PI4 = 2.3561944901923448  # 3*pi/4, subtracted from x before reduction
# range reduction: k = round(xp/(2pi)); theta = xp - k*2pi (Cody-Waite split)
INV_2PI = 0.15915494309189535
MAGIC = 12582912.0  # 1.5 * 2^23
TWO_PI = 6.283185307179586
CW1 = 6.28125
CW2 = float(np.float32(TWO_PI - CW1))
CW3 = float(np.float32(TWO_PI - CW1 - np.float64(np.float32(TWO_PI - CW1))))
# rsqrt seed: sqrt(v) ~ Q2*v^2 + Q1*v + Q0 on v=1/x in [1/30, 1/8] (6e-3 rel),
# then two Newton steps vs x.
RSQ_Q2, RSQ_Q1, RSQ_Q0 = -6.530129277046191, 2.874979928348082, 0.09510543429983853
# sin(theta) = theta * p(s), s = theta^2; p deg-7 minimax on [0, pi^2]
SIN_CH = (
    (-6.594788005314608e-13, 1.5892518819899321e-10, -2.5038348578065054e-08),  # top
    (2.755669345253912e-06, -0.00019841254455000887, 0.0),
    (0.00833333313975342, -0.1666666665537394, 1.0000000000445453),
)

P = 128
COLS = 65536          # per-core elements / 128 partitions
FREE = 1024           # tile free dim
NT = COLS // FREE
N_CORES = 8

_CACHE: dict = {}


def _register_custom_ops():
    from concourse import dve_ops
    from concourse.dve_spec import Spec, Src0, Src1, C0, C1, C2, sq, lower, _has_src1
    from concourse.dve_uop import DveOpSpec

    def register_op(name, spec):
        for op in dve_ops.OPS:
            if op.name == name:
                return op
        row = max(dve_ops._SUB_OPCODE_FOR_NAME.values()) + 1
        assert row < 0x20, "out of custom-DVE opcode rows"
        dve_ops._SUB_OPCODE_FOR_NAME[name] = row
        shas = {}
        for ver in ("v3", "v4"):
            try:
                s = DveOpSpec(name=name, opcode=row, uops=lower(spec, ver=ver),
                              rd1_en=_has_src1(spec))
                shas[ver] = s.sha(ver)
            except Exception:
                if ver == "v3":
                    raise
        op = dve_ops.DveOp(name, spec, subdim=False, uops_sha=shas)
        dve_ops.OPS.append(op)
        dve_ops.CUSTOM_DVE_SPECS[name] = spec
        return op

    ops = {}
    ops["POLY3T"] = register_op("J2_POLY3T", Spec(
        body=((C0 * Src0 + C1) * Src0 + C2) * Src0,
        reference=lambda in0, in1, c0, c1, c2: ((c0 * in0 + c1) * in0 + c2) * in0,
    ))
    ops["HORNER2"] = register_op("J2_HORNER2", Spec(
        body=((Src0 + C0) * Src1 + C1) * Src1 + C2,
        reference=lambda in0, in1, c0, c1, c2: ((in0 + c0) * in1 + c1) * in1 + c2,
    ))
    ops["PHASE"] = register_op("J2_PHASE", Spec(
        body=Src1 + ((C0 * sq(Src0) + C1) * sq(Src0) + C2) * Src0,
        reference=lambda in0, in1, c0, c1, c2:
            in1 + ((c0 * in0 * in0 + c1) * (in0 * in0) + c2) * in0,
    ))
    ops["AMP3"] = register_op("J2_AMP3", Spec(
        body=((C0 * sq(Src0) + C1) * sq(Src0) + C2) * sq(Src0),
        reference=lambda in0, in1, c0, c1, c2:
            ((c0 * in0 * in0 + c1) * (in0 * in0) + c2) * (in0 * in0),
    ))
    ops["MADD"] = register_op("J2_MADD", Spec(
        body=(Src0 + C0) * Src1,
        reference=lambda in0, in1, c0, c1, c2: (in0 + c0) * in1,
    ))
    ops["RSQRT_NR"] = register_op("J2_RSQRT_NR", Spec(
        body=(C0 - sq(Src0) * Src1) * Src0 * C1,
        reference=lambda in0, in1, c0, c1, c2: (c0 - in0 * in0 * in1) * in0 * c1,
    ))
    ops["QUAD"] = register_op("J2_QUAD", Spec(
        body=(C0 * Src0 + C1) * Src0 + C2,
        reference=lambda in0, in1, c0, c1, c2: (c0 * in0 + c1) * in0 + c2,
    ))
    ops["TVAR"] = register_op("J2_TVAR", Spec(
        body=sq(Src0) * C0 + C1,
        reference=lambda in0, in1, c0, c1, c2: in0 * in0 * c0 + c1,
    ))
    ops["ROUND"] = register_op("J2_ROUND", Spec(
        body=(Src0 * C0 + C2) - C2,
        reference=lambda in0, in1, c0, c1, c2:
            np.float32(np.float32(in0 * np.float32(c0)) + np.float32(c2))
            - np.float32(c2),
    ))
    ops["H2SQ"] = register_op("J2_H2SQ", Spec(
        body=((Src0 + C0) * sq(Src1) + C1) * sq(Src1) + C2,
        reference=lambda in0, in1, c0, c1, c2:
            ((in0 + c0) * (in1 * in1) + c1) * (in1 * in1) + c2,
    ))
    ops["H2SQM"] = register_op("J2_H2SQM", Spec(
        body=(((Src0 + C0) * sq(Src1) + C1) * sq(Src1) + C2) * Src1,
        reference=lambda in0, in1, c0, c1, c2:
            (((in0 + c0) * (in1 * in1) + c1) * (in1 * in1) + c2) * in1,
    ))
    return ops


def _build_program(repeat: int = 1, free: int = FREE):
    key = (repeat, free)
    if key in _CACHE:
        return _CACHE[key]

    from contextlib import ExitStack, nullcontext

    import concourse.bacc as bacc
    import concourse.bass as bass
    import concourse.tile as tile
    from concourse import mybir

    ops = _register_custom_ops()
    f32 = mybir.dt.float32
    ALU = mybir.AluOpType
    nt = COLS // free

    nc = bacc.Bacc("TRN2", target_bir_lowering=False, debug=False)
    x_d = nc.dram_tensor("x", [P, COLS], f32, kind="ExternalInput")
    o_d = nc.dram_tensor("out", [P, COLS], f32, kind="ExternalOutput")
    x_ap = x_d.ap()
    o_ap = o_d.ap()

    cd = nc.vector._custom_dve

    with tile.TileContext(nc) as tc, ExitStack() as ctx:
        pools = {}
        for name in ("xt", "mk", "xs", "tv", "w", "rf", "q0", "r1", "rs",
                     "xp", "kk", "th", "s1", "s2", "sb", "kt", "aa", "ot"):
            pools[name] = ctx.enter_context(tc.tile_pool(name=name, bufs=2))

        def pt(pool, tag=None, dtype=None):
            return pools[pool].tile([P, free], dtype or f32, name=tag or pool,
                                    tag=tag or pool)

        loop_cm = tc.For_i(0, repeat, 1) if repeat > 1 else nullcontext()
        with loop_cm:
          for i in range(nt):
            sl = bass.ts(i, free)
            xt = pt("xt")
            nc.sync.dma_start(xt[:], x_ap[:, sl])

            # mask = (x < 8) as uint8 1/0
            mk = pt("mk", dtype=mybir.dt.uint8)
            nc.vector.tensor_scalar(mk[:], xt[:], 8.0, None, ALU.is_lt)

            # --- small branch: deg-9 poly in t = x^2/32 - 1 ---
            tv = pt("tv")
            cd(ops["TVAR"], out=tv[:], in0=xt[:], s0=1.0 / 32.0, s1=-1.0)
            w = pt("w", tag="w0")
            cd(ops["POLY3T"], out=w[:], in0=tv[:], s0=SM_A0, s1=SM_A1, imm2=SM_A2)
            for j, (a_, b_, c_) in enumerate(SM_CH):
                w2 = pt("w", tag=f"w{j + 1}")
                cd(ops["HORNER2"], out=w2[:], in0=w[:], in1=tv[:],
                   s0=a_, s1=b_, imm2=c_)
                w = w2
            small = w

            # --- big branch (x >= 8 live; other lanes are dead garbage) ---
            rf = pt("rf")
            nc.vector.reciprocal_approx_fast(out=rf[:], in_=xt[:])
            q0 = pt("q0")
            cd(ops["QUAD"], out=q0[:], in0=rf[:], s0=RSQ_Q2, s1=RSQ_Q1, imm2=RSQ_Q0)
            r1 = pt("r1")
            cd(ops["RSQRT_NR"], out=r1[:], in0=q0[:], in1=xt[:], s0=3.0, s1=0.5)
            rs = pt("rs")
            cd(ops["RSQRT_NR"], out=rs[:], in0=r1[:], in1=xt[:], s0=3.0, s1=0.5)

            xs = pt("xs")
            nc.vector.tensor_scalar(xs[:], xt[:], BIAS_3PI4, None, ALU.subtract)
            xp = pt("xp")
            cd(ops["PHASE"], out=xp[:], in0=rf[:], in1=xs[:], s0=G2, s1=G1, imm2=G0)
            kk = pt("kk")
            cd(ops["ROUND"], out=kk[:], in0=xp[:], s0=INV_2PI, imm2=MAGIC)
            th = pt("th")
            nc.vector.cody_waite_cascade(th[:], xp[:], kk[:], CW1, CW2, CW3)

            # sin(theta) = theta * p(theta^2), deg-7 p
            s1_ = pt("s1")
            cd(ops["AMP3"], out=s1_[:], in0=th[:],
               s0=SIN_CH[0][0], s1=SIN_CH[0][1], imm2=SIN_CH[0][2])
            s2 = pt("s2")
            cd(ops["H2SQ"], out=s2[:], in0=s1_[:], in1=th[:],
               s0=SIN_CH[1][0], s1=SIN_CH[1][1], imm2=SIN_CH[1][2])
            sb = pt("sb")
            cd(ops["H2SQM"], out=sb[:], in0=s2[:], in1=th[:],
               s0=SIN_CH[2][0], s1=SIN_CH[2][1], imm2=SIN_CH[2][2])

            kt = pt("kt")
            cd(ops["AMP3"], out=kt[:], in0=rf[:], s0=K3, s1=K2, imm2=K1)
            aa = pt("aa")
            cd(ops["MADD"], out=aa[:], in0=kt[:], in1=rs[:], s0=K0)
            ot = pt("ot")
            nc.vector.tensor_tensor(ot[:], aa[:], sb[:], ALU.mult)

            nc.vector.copy_predicated(ot[:], mk[:], small[:])
            nc.sync.dma_start(o_ap[:, sl], ot[:])

    nc.compile()
    _CACHE[key] = {"nc": nc}
    return _CACHE[key]


def kernel(x: np.ndarray) -> np.ndarray:
    from concourse import bass_utils

    prog = _build_program()
    x = np.asarray(x, dtype=np.float32)
    rows = x.shape[0] // N_CORES
    in_maps = [
        {"x": np.ascontiguousarray(
            x[rows * k: rows * (k + 1)].reshape(P, COLS))}
        for k in range(N_CORES)
    ]
    res = bass_utils.run_bass_kernel_spmd(
        prog["nc"], in_maps, core_ids=list(range(N_CORES)))
    out = np.concatenate(
        [res.results[k]["out"].reshape(rows, -1) for k in range(N_CORES)], axis=0)
    return out.astype(np.float32)

